# revision 22
# baseline (speedup 1.0000x reference)
"""MoE block (8 experts, top-2) on 8 Trainium2 NeuronCores.

Strategy: expert parallelism. The gate (x @ Wg + bg, 0.01% of total FLOPs)
plus top-2 routing runs on the host as part of the sharding step; each of
the 8 cores then runs one expert's FFN over that expert's tokens:

    yT_e = (relu(X_e @ W1[e] + b1[e]) @ W2[e] + b2[e])^T

Device-side layout keeps activations transposed ([feature, token]) so both
matmuls use natural weight layouts as the stationary operand:

    H^T = W1^T X^T   (contract d=1024,  8 k-tiles)
    Y^T = W2^T H^T   (contract dff=4096, 32 k-tiles)

Operands are bf16 (f32 PSUM accumulation): same 1-col/cycle PE rate as
f32r but LDWEIGHTS gets FWL (hides under the matmul stream) and all HBM
traffic halves. End-to-end rel err ~5e-4, far inside the 2e-2 gate.
X^T and H^T stay resident in SBUF for the whole token capacity while W1
and W2 each stream from HBM exactly once (phase-split). X^T lands as one
DMA per token block so the first matmuls start ~2 us in; W2 streams on
the gpsimd (SWDGE) queue so its prefetch overlaps the tail of the W1
stream. The host applies the top-2 softmax weights and scatters back.
"""

import numpy as np
import ml_dtypes

import concourse.bacc as bacc
import concourse.mybir as mybir
from concourse.tile import TileContext
from concourse.bass_utils import run_bass_kernel_spmd

D = 1024
DFF = 4096
E = 8
TOPK = 2
KD = D // 128      # 8   k-tiles for phase 1
MF = DFF // 128    # 32  dff tiles (phase-1 output / phase-2 contraction)
KF = DFF // 128    # 32
MD = D // 128      # 8   output d tiles

F32 = mybir.dt.float32
BF16 = mybir.dt.bfloat16
BF16NP = ml_dtypes.bfloat16

# Per-partition SBUF: hts 64*C B + xblk 16*C B + resident W2 64 KiB +
# W1 ring/consts ~10 KiB must fit ~208 KiB usable.
MAX_SINGLEPASS_C = 1536

_KERNEL_CACHE = {}


def _build_singlepass(C, NB, nblk):
    """Per-core program, whole capacity resident: phase 1 (stream W1 once)
    then phase 2 (stream W2 once, prefetched on the SWDGE queue)."""
    assert nblk * NB == C

    nc = bacc.Bacc(None, target_bir_lowering=False)
    # Block 0 is k-major so each [128, NB] k-slice is one contiguous read;
    # later blocks are partition-major for single whole-block DMAs.
    xT0 = nc.dram_tensor("xT0", [KD, 128, NB], BF16, kind="ExternalInput")
    xTb = nc.dram_tensor("xTb", [nblk - 1, 128, KD, NB], BF16,
                         kind="ExternalInput")
    w1 = nc.dram_tensor("w1", [MF, 128, KD, 128], BF16, kind="ExternalInput")
    b1c = nc.dram_tensor("b1c", [128, MF], F32, kind="ExternalInput")
    w2 = nc.dram_tensor("w2", [MD, 128, KF, 128], BF16, kind="ExternalInput")
    b2c = nc.dram_tensor("b2c", [128, MD], F32, kind="ExternalInput")
    yT = nc.dram_tensor("yT", [D, C], BF16, kind="ExternalOutput")

    with TileContext(nc) as tc:
        with (
            tc.tile_pool(name="acts", bufs=1) as acts,
            tc.tile_pool(name="wpool", bufs=1) as wpool,
            tc.tile_pool(name="cpool", bufs=1) as cpool,
            tc.tile_pool(name="opool", bufs=1) as opool,
            tc.tile_pool(name="psum", bufs=8, space="PSUM") as psum,
        ):
            # biases via gpsimd (SWDGE) — keeps both HWDGE queues free for
            # the latency-critical streams.
            b1t = cpool.tile([128, MF], F32, name="b1t")
            nc.gpsimd.dma_start(out=b1t[:], in_=b1c[:])
            b2t = cpool.tile([128, MD], F32, name="b2t")
            nc.gpsimd.dma_start(out=b2t[:], in_=b2c[:])

            MCH = 4

            # Startup choreography: X block 0 lands as 8 contiguous per-k
            # slice DMAs, interleaved across the scalar and sync HWDGE
            # engines (each dma_start occupies its engine ~650 ns, so one
            # engine alone would gate block 0 on trigger issue). The sync
            # engine leads with W1 tiles for the first m-chunk, paced so
            # each is there just before its m-tile's matmuls.
            xs0 = [None] * KD
            w1_pre = []

            def w1_dma(m):
                w1t = wpool.tile([128, KD, 128], BF16, name="w1t",
                                 tag="w1t", bufs=2 * MCH)
                nc.sync.dma_start(out=w1t[:], in_=w1[m])
                return w1t

            def x0_dma(k, eng):
                t = acts.tile([128, NB], BF16, name=f"x0_{k}", tag=f"x0_{k}")
                eng.dma_start(out=t[:], in_=xT0[k])
                xs0[k] = t

            w1_pre.append(w1_dma(0))          # sync
            for k in (0, 2, 4, 6):
                x0_dma(k, nc.scalar)          # scalar
            x0_dma(1, nc.sync)
            w1_pre.append(w1_dma(1))
            x0_dma(3, nc.sync)
            w1_pre.append(w1_dma(2))
            x0_dma(5, nc.sync)
            w1_pre.append(w1_dma(3))
            x0_dma(7, nc.sync)

            # Blocks 1+ as whole-block DMAs on the scalar queue, then the
            # SBUF-resident W2 tiles (8 x 1 MB) behind them: the 8 DMA
            # completion sem lanes round-robin, so each W2 trigger waits
            # for earlier X traffic — deferring the W2 stream until the
            # startup burst is done.
            xbs = []
            for nb in range(1, nblk):
                t = acts.tile([128, KD, NB], BF16, name=f"xb{nb}",
                              tag=f"xb{nb}")
                nc.scalar.dma_start(out=t[:], in_=xTb[nb - 1])
                xbs.append(t)

            def xrhs(nb, k):
                return xs0[k][:] if nb == 0 else xbs[nb - 1][:, k, :]

            w2ts = []
            for mo in range(MD):
                t = wpool.tile([128, KF, 128], BF16, name=f"w2_{mo}",
                               tag=f"w2_{mo}")
                nc.scalar.dma_start(out=t[:], in_=w2[mo])
                w2ts.append(t)

            hts = [
                acts.tile([128, C], BF16, name=f"ht{m}", tag=f"ht{m}")
                for m in range(MF)
            ]

            # phase 1: H^T[m] = relu(sum_k W1[k,m]^T @ X^T[k] + b1[m])
            # W1 tile for m streams as one 256 KB DMA on the sync queue.
            # m advances in chunks with the block loop outside, so early
            # matmuls ride the arriving X stream (block 0 serves a chunk
            # of m-tiles of work before block 1 is needed).
            for mc in range(0, MF, MCH):
                if mc == 0:
                    w1ts = w1_pre
                else:
                    w1ts = []
                    for m in range(mc, mc + MCH):
                        w1t = wpool.tile([128, KD, 128], BF16, name="w1t",
                                         tag="w1t", bufs=2 * MCH)
                        nc.sync.dma_start(out=w1t[:], in_=w1[m])
                        w1ts.append(w1t)
                for nb in range(nblk):
                    ns = slice(nb * NB, (nb + 1) * NB)
                    for m in range(mc, mc + MCH):
                        ps = psum.tile([128, 512], F32, name="ps",
                                       tag="ps")[:, :NB]
                        for k in range(KD):
                            nc.tensor.matmul(
                                ps, lhsT=w1ts[m - mc][:, k, :],
                                rhs=xrhs(nb, k),
                                start=(k == 0), stop=(k == KD - 1),
                            )
                        nc.scalar.activation(
                            hts[m][:, ns], ps,
                            mybir.ActivationFunctionType.Relu,
                            bias=b1t[:, m:m + 1],
                        )

            # phase 2: Y^T[mo] = sum_k W2[k,mo]^T @ H^T[k] + b2[mo]
            for mo in range(MD):
                w2t = w2ts[mo]
                for nb in range(nblk):
                    ns = slice(nb * NB, (nb + 1) * NB)
                    ps = psum.tile([128, 512], F32, name="ps2", tag="ps")[:, :NB]
                    for k in range(KF):
                        nc.tensor.matmul(
                            ps, lhsT=w2t[:, k, :], rhs=hts[k][:, ns],
                            start=(k == 0), stop=(k == KF - 1),
                        )
                    ot = opool.tile([128, NB], BF16, name="ot", tag="ot", bufs=4)
                    nc.scalar.activation(
                        ot[:], ps,
                        mybir.ActivationFunctionType.Identity,
                        bias=b2t[:, mo:mo + 1],
                    )
                    nc.scalar.dma_start(
                        out=yT[mo * 128:(mo + 1) * 128, ns], in_=ot[:]
                    )
    nc.compile()
    return nc


def _plan(maxc):
    """Pick capacity/tiling. Blocks must be <= 512 (one PSUM bank of f32)."""
    nblk = max(1, -(-maxc // 512))
    NB = max(256, -(-maxc // nblk))
    C = nblk * NB
    return ("single", C, NB, nblk)


def _get_kernel(plan):
    if plan not in _KERNEL_CACHE:
        kind, C, NB, nblk = plan
        _KERNEL_CACHE[plan] = _build_singlepass(C, NB, nblk)
    return _KERNEL_CACHE[plan]


def kernel(x, Wg, bg, W1, b1, W2, b2):
    x = np.asarray(x, dtype=np.float32)
    Wg = np.asarray(Wg, dtype=np.float32)
    bg = np.asarray(bg, dtype=np.float32)
    W1 = np.asarray(W1, dtype=np.float32)
    b1 = np.asarray(b1, dtype=np.float32)
    W2 = np.asarray(W2, dtype=np.float32)
    b2 = np.asarray(b2, dtype=np.float32)

    fsz = x.shape[:-1]
    xf = x.reshape(-1, D)
    n = xf.shape[0]

    # ---- routing (host): gate -> top-2 -> softmax over the top-2 ----
    gate = xf @ Wg + bg                                   # [N, E] f32
    top2 = np.argsort(-gate, axis=-1, kind="stable")[:, :TOPK]   # desc, ties->low idx
    vals = np.take_along_axis(gate, top2, axis=-1)        # [N, 2] sorted desc
    ex = np.exp(vals - vals[:, :1])
    wts = ex / ex.sum(axis=-1, keepdims=True)             # [N, 2] f32

    idx_lists = []
    wt_lists = []
    counts = np.zeros(E, dtype=np.int64)
    for e in range(E):
        tok, slot = np.nonzero(top2 == e)
        idx_lists.append(tok)
        wt_lists.append(wts[tok, slot])
        counts[e] = tok.shape[0]
    maxc = int(counts.max())

    plan = _plan(maxc)
    _, C, NB, nblk = plan
    assert C <= MAX_SINGLEPASS_C, "capacity beyond single-pass SBUF budget"
    nc = _get_kernel(plan)

    # ---- shard: gather tokens + pre-tile weights per expert ----
    in_maps = []
    for e in range(E):
        xe = np.zeros((C, D), dtype=np.float32)
        xe[:counts[e]] = xf[idx_lists[e]]
        xT = xe.T                                           # [D, C]
        xTk = xT.reshape(KD, 128, nblk, NB)
        xT0 = np.ascontiguousarray(xTk[:, :, 0, :]).astype(BF16NP)
        xTb = np.ascontiguousarray(
            xTk[:, :, 1:, :].transpose(2, 1, 0, 3)
        ).astype(BF16NP)                                    # [nblk-1,128,KD,NB]
        w1h = np.ascontiguousarray(
            W1[e].reshape(KD, 128, MF, 128).transpose(2, 1, 0, 3)
        ).astype(BF16NP)                                    # [MF,128,KD,128]
        w2h = np.ascontiguousarray(
            W2[e].reshape(KF, 128, MD, 128).transpose(2, 1, 0, 3)
        ).astype(BF16NP)                                    # [MD,128,KF,128]
        b1c = np.ascontiguousarray(b1[e].reshape(MF, 128).T)  # [128, MF]
        b2c = np.ascontiguousarray(b2[e].reshape(MD, 128).T)  # [128, MD]
        in_maps.append(
            {"xT0": xT0, "xTb": xTb, "w1": w1h, "b1c": b1c, "w2": w2h,
             "b2c": b2c}
        )

    res = run_bass_kernel_spmd(nc, in_maps, core_ids=list(range(E)))

    # ---- combine (host): apply top-2 softmax weights, scatter-add ----
    out = np.zeros((n, D), dtype=np.float32)
    for e in range(E):
        ye = res.results[e]["yT"].astype(np.float32).T[:counts[e]]  # [count, D]
        out[idx_lists[e]] += wt_lists[e][:, None] * ye
    return out.reshape(*fsz, D)


# revision 28
# speedup vs baseline: 1.0270x; 1.0270x over previous
"""MoE block (8 experts, top-2) on 8 Trainium2 NeuronCores.

Strategy: expert parallelism. The gate (x @ Wg + bg, 0.01% of total FLOPs)
plus top-2 routing runs on the host as part of the sharding step; each of
the 8 cores then runs one expert's FFN over that expert's tokens:

    yT_e = (relu(X_e @ W1[e] + b1[e]) @ W2[e] + b2[e])^T

Device-side layout keeps activations transposed ([feature, token]) so both
matmuls use natural weight layouts as the stationary operand:

    H^T = W1^T X^T   (contract d=1024,  8 k-tiles)
    Y^T = W2^T H^T   (contract dff=4096, 32 k-tiles)

Operands are bf16 (f32 PSUM accumulation): same 1-col/cycle PE rate as
f32r but LDWEIGHTS gets FWL (hides under the matmul stream) and all HBM
traffic halves. End-to-end rel err ~5e-4, far inside the 2e-2 gate.
X^T and H^T stay resident in SBUF for the whole token capacity while W1
and W2 each stream from HBM exactly once (phase-split). X^T lands as one
DMA per token block so the first matmuls start ~2 us in; W2 streams on
the gpsimd (SWDGE) queue so its prefetch overlaps the tail of the W1
stream. The host applies the top-2 softmax weights and scatters back.
"""

import numpy as np
import ml_dtypes

import concourse.bacc as bacc
import concourse.mybir as mybir
from concourse.tile import TileContext
from concourse.bass_utils import run_bass_kernel_spmd

D = 1024
DFF = 4096
E = 8
TOPK = 2
KD = D // 128      # 8   k-tiles for phase 1
MF = DFF // 128    # 32  dff tiles (phase-1 output / phase-2 contraction)
KF = DFF // 128    # 32
MD = D // 128      # 8   output d tiles

F32 = mybir.dt.float32
BF16 = mybir.dt.bfloat16
BF16NP = ml_dtypes.bfloat16

# Per-partition SBUF: hts 64*C B + xblk 16*C B + resident W2 64 KiB +
# W1 ring/consts ~10 KiB must fit ~208 KiB usable.
MAX_SINGLEPASS_C = 1536

_KERNEL_CACHE = {}


def _build_singlepass(C, NB, nblk):
    """Per-core program, whole capacity resident: phase 1 (stream W1 once)
    then phase 2 (stream W2 once, prefetched on the SWDGE queue)."""
    assert nblk * NB == C

    nc = bacc.Bacc(None, target_bir_lowering=False)
    # Block 0 is split into k-pairs (1456 B per partition per DMA — small
    # runs below ~1 KiB/partition collapse to ~20 GB/s); later blocks are
    # partition-major for single whole-block DMAs.
    xT0 = nc.dram_tensor("xT0", [KD // 2, 128, 2, NB], BF16,
                         kind="ExternalInput")
    xTb = nc.dram_tensor("xTb", [nblk - 1, 128, KD, NB], BF16,
                         kind="ExternalInput")
    w1 = nc.dram_tensor("w1", [MF, 128, KD, 128], BF16, kind="ExternalInput")
    b1c = nc.dram_tensor("b1c", [128, MF], F32, kind="ExternalInput")
    w2 = nc.dram_tensor("w2", [MD, 128, KF, 128], BF16, kind="ExternalInput")
    b2c = nc.dram_tensor("b2c", [128, MD], F32, kind="ExternalInput")
    yT = nc.dram_tensor("yT", [D, C], BF16, kind="ExternalOutput")

    with TileContext(nc) as tc:
        with (
            tc.tile_pool(name="acts", bufs=1) as acts,
            tc.tile_pool(name="wpool", bufs=1) as wpool,
            tc.tile_pool(name="cpool", bufs=1) as cpool,
            tc.tile_pool(name="opool", bufs=1) as opool,
            tc.tile_pool(name="psum", bufs=8, space="PSUM") as psum,
        ):
            # biases via gpsimd (SWDGE) — keeps both HWDGE queues free for
            # the latency-critical streams.
            b1t = cpool.tile([128, MF], F32, name="b1t")
            nc.gpsimd.dma_start(out=b1t[:], in_=b1c[:])
            b2t = cpool.tile([128, MD], F32, name="b2t")
            nc.gpsimd.dma_start(out=b2t[:], in_=b2c[:])

            MCH = 4

            # Startup choreography: X block 0 lands as 4 k-pair DMAs
            # interleaved across the scalar and sync HWDGE engines (each
            # dma_start occupies its engine ~650 ns, so one engine alone
            # would gate block 0 on trigger issue). The sync engine leads
            # with the first W1 tiles.
            xp = [None] * (KD // 2)
            w1_pre = []

            def w1_dma(m):
                w1t = wpool.tile([128, KD, 128], BF16, name="w1t",
                                 tag="w1t", bufs=2 * MCH)
                nc.sync.dma_start(out=w1t[:], in_=w1[m])
                return w1t

            def x0_dma(j, eng):
                t = acts.tile([128, 2, NB], BF16, name=f"x0_{j}",
                              tag=f"x0_{j}")
                eng.dma_start(out=t[:], in_=xT0[j])
                xp[j] = t

            w1_pre.append(w1_dma(0))          # sync
            x0_dma(0, nc.scalar)
            x0_dma(1, nc.sync)
            x0_dma(2, nc.scalar)
            x0_dma(3, nc.sync)
            w1_pre.append(w1_dma(1))
            # Blocks 1+ as whole-block DMAs on the scalar queue.
            xbs = []
            for nb in range(1, nblk):
                t = acts.tile([128, KD, NB], BF16, name=f"xb{nb}",
                              tag=f"xb{nb}")
                nc.scalar.dma_start(out=t[:], in_=xTb[nb - 1])
                xbs.append(t)
            w1_pre.append(w1_dma(2))
            w1_pre.append(w1_dma(3))

            def xrhs(nb, k):
                if nb == 0:
                    return xp[k // 2][:, k % 2, :]
                return xbs[nb - 1][:, k, :]

            # W2 destination tiles (SBUF-resident, 8 x 1 MB). Their DMAs
            # are emitted inside the phase-1 chunk loop below, so priority
            # order spreads the W2 stream across phase 1 instead of
            # fighting the startup X burst.
            w2ts = [
                wpool.tile([128, KF, 128], BF16, name=f"w2_{mo}",
                           tag=f"w2_{mo}")
                for mo in range(MD)
            ]

            hts = [
                acts.tile([128, C], BF16, name=f"ht{m}", tag=f"ht{m}")
                for m in range(MF)
            ]

            # phase 1: H^T[m] = relu(sum_k W1[k,m]^T @ X^T[k] + b1[m])
            # W1 tile for m streams as one 256 KB DMA on the sync queue.
            # m advances in chunks with the block loop outside, so early
            # matmuls ride the arriving X stream (block 0 serves a chunk
            # of m-tiles of work before block 1 is needed).
            for mc in range(0, MF, MCH):
                if mc == 0:
                    w1ts = w1_pre
                else:
                    w1ts = [w1_dma(m) for m in range(mc, mc + MCH)]
                    # One W2 tile streams per chunk (scalar queue): spread
                    # evenly over phase 1, clear of the startup burst.
                    mo = mc // MCH - 1
                    nc.scalar.dma_start(out=w2ts[mo][:], in_=w2[mo])
                for nb in range(nblk):
                    ns = slice(nb * NB, (nb + 1) * NB)
                    for m in range(mc, mc + MCH):
                        ps = psum.tile([128, 512], F32, name="ps",
                                       tag="ps")[:, :NB]
                        for k in range(KD):
                            nc.tensor.matmul(
                                ps, lhsT=w1ts[m - mc][:, k, :],
                                rhs=xrhs(nb, k),
                                start=(k == 0), stop=(k == KD - 1),
                            )
                        nc.scalar.activation(
                            hts[m][:, ns], ps,
                            mybir.ActivationFunctionType.Relu,
                            bias=b1t[:, m:m + 1],
                        )

            nc.scalar.dma_start(out=w2ts[MD - 1][:], in_=w2[MD - 1])

            # phase 2: Y^T[mo] = sum_k W2[k,mo]^T @ H^T[k] + b2[mo]
            # One output tile per mo: the three block activations fill it,
            # then a single DMA (2184 B/partition — fat enough for full
            # DMA rate) stores the whole row block.
            for mo in range(MD):
                w2t = w2ts[mo]
                ot = opool.tile([128, C], BF16, name="ot", tag="ot", bufs=2)
                for nb in range(nblk):
                    ns = slice(nb * NB, (nb + 1) * NB)
                    ps = psum.tile([128, 512], F32, name="ps2", tag="ps")[:, :NB]
                    for k in range(KF):
                        nc.tensor.matmul(
                            ps, lhsT=w2t[:, k, :], rhs=hts[k][:, ns],
                            start=(k == 0), stop=(k == KF - 1),
                        )
                    nc.scalar.activation(
                        ot[:, ns], ps,
                        mybir.ActivationFunctionType.Identity,
                        bias=b2t[:, mo:mo + 1],
                    )
                nc.scalar.dma_start(
                    out=yT[mo * 128:(mo + 1) * 128, :], in_=ot[:]
                )
    nc.compile()
    return nc


def _plan(maxc):
    """Pick capacity/tiling. Blocks must be <= 512 (one PSUM bank of f32)."""
    nblk = max(1, -(-maxc // 512))
    NB = max(256, -(-maxc // nblk))
    C = nblk * NB
    return ("single", C, NB, nblk)


def _get_kernel(plan):
    if plan not in _KERNEL_CACHE:
        kind, C, NB, nblk = plan
        _KERNEL_CACHE[plan] = _build_singlepass(C, NB, nblk)
    return _KERNEL_CACHE[plan]


def kernel(x, Wg, bg, W1, b1, W2, b2):
    x = np.asarray(x, dtype=np.float32)
    Wg = np.asarray(Wg, dtype=np.float32)
    bg = np.asarray(bg, dtype=np.float32)
    W1 = np.asarray(W1, dtype=np.float32)
    b1 = np.asarray(b1, dtype=np.float32)
    W2 = np.asarray(W2, dtype=np.float32)
    b2 = np.asarray(b2, dtype=np.float32)

    fsz = x.shape[:-1]
    xf = x.reshape(-1, D)
    n = xf.shape[0]

    # ---- routing (host): gate -> top-2 -> softmax over the top-2 ----
    gate = xf @ Wg + bg                                   # [N, E] f32
    top2 = np.argsort(-gate, axis=-1, kind="stable")[:, :TOPK]   # desc, ties->low idx
    vals = np.take_along_axis(gate, top2, axis=-1)        # [N, 2] sorted desc
    ex = np.exp(vals - vals[:, :1])
    wts = ex / ex.sum(axis=-1, keepdims=True)             # [N, 2] f32

    idx_lists = []
    wt_lists = []
    counts = np.zeros(E, dtype=np.int64)
    for e in range(E):
        tok, slot = np.nonzero(top2 == e)
        idx_lists.append(tok)
        wt_lists.append(wts[tok, slot])
        counts[e] = tok.shape[0]
    maxc = int(counts.max())

    plan = _plan(maxc)
    _, C, NB, nblk = plan
    assert C <= MAX_SINGLEPASS_C, "capacity beyond single-pass SBUF budget"
    nc = _get_kernel(plan)

    # ---- shard: gather tokens + pre-tile weights per expert ----
    in_maps = []
    for e in range(E):
        xe = np.zeros((C, D), dtype=np.float32)
        xe[:counts[e]] = xf[idx_lists[e]]
        xT = xe.T                                           # [D, C]
        xTk = xT.reshape(KD // 2, 2, 128, nblk, NB)
        xT0 = np.ascontiguousarray(
            xTk[:, :, :, 0, :].transpose(0, 2, 1, 3)
        ).astype(BF16NP)                                    # [KD/2,128,2,NB]
        xTb = np.ascontiguousarray(
            xTk[:, :, :, 1:, :].transpose(3, 2, 0, 1, 4)
        ).reshape(nblk - 1, 128, KD, NB).astype(BF16NP)     # [nblk-1,128,KD,NB]
        w1h = np.ascontiguousarray(
            W1[e].reshape(KD, 128, MF, 128).transpose(2, 1, 0, 3)
        ).astype(BF16NP)                                    # [MF,128,KD,128]
        w2h = np.ascontiguousarray(
            W2[e].reshape(KF, 128, MD, 128).transpose(2, 1, 0, 3)
        ).astype(BF16NP)                                    # [MD,128,KF,128]
        b1c = np.ascontiguousarray(b1[e].reshape(MF, 128).T)  # [128, MF]
        b2c = np.ascontiguousarray(b2[e].reshape(MD, 128).T)  # [128, MD]
        in_maps.append(
            {"xT0": xT0, "xTb": xTb, "w1": w1h, "b1c": b1c, "w2": w2h,
             "b2c": b2c}
        )

    res = run_bass_kernel_spmd(nc, in_maps, core_ids=list(range(E)))

    # ---- combine (host): apply top-2 softmax weights, scatter-add ----
    out = np.zeros((n, D), dtype=np.float32)
    for e in range(E):
        ye = res.results[e]["yT"].astype(np.float32).T[:counts[e]]  # [count, D]
        out[idx_lists[e]] += wt_lists[e][:, None] * ye
    return out.reshape(*fsz, D)


# revision 30
# speedup vs baseline: 1.0305x; 1.0033x over previous
"""MoE block (8 experts, top-2) on 8 Trainium2 NeuronCores.

Strategy: expert parallelism. The gate (x @ Wg + bg, 0.01% of total FLOPs)
plus top-2 routing runs on the host as part of the sharding step; each of
the 8 cores then runs one expert's FFN over that expert's tokens:

    yT_e = (relu(X_e @ W1[e] + b1[e]) @ W2[e] + b2[e])^T

Device-side layout keeps activations transposed ([feature, token]) so both
matmuls use natural weight layouts as the stationary operand:

    H^T = W1^T X^T   (contract d=1024,  8 k-tiles)
    Y^T = W2^T H^T   (contract dff=4096, 32 k-tiles)

Operands are bf16 (f32 PSUM accumulation): same 1-col/cycle PE rate as
f32r but LDWEIGHTS gets FWL (hides under the matmul stream) and all HBM
traffic halves. End-to-end rel err ~5e-4, far inside the 2e-2 gate.
X^T and H^T stay resident in SBUF for the whole token capacity while W1
and W2 each stream from HBM exactly once (phase-split). X^T lands as one
DMA per token block so the first matmuls start ~2 us in; W2 streams on
the gpsimd (SWDGE) queue so its prefetch overlaps the tail of the W1
stream. The host applies the top-2 softmax weights and scatters back.
"""

import numpy as np
import ml_dtypes

import concourse.bacc as bacc
import concourse.mybir as mybir
from concourse.tile import TileContext
from concourse.bass_utils import run_bass_kernel_spmd

D = 1024
DFF = 4096
E = 8
TOPK = 2
KD = D // 128      # 8   k-tiles for phase 1
MF = DFF // 128    # 32  dff tiles (phase-1 output / phase-2 contraction)
KF = DFF // 128    # 32
MD = D // 128      # 8   output d tiles

F32 = mybir.dt.float32
BF16 = mybir.dt.bfloat16
BF16NP = ml_dtypes.bfloat16

# Per-partition SBUF: hts 64*C B + xblk 16*C B + resident W2 64 KiB +
# W1 ring/consts ~10 KiB must fit ~208 KiB usable.
MAX_SINGLEPASS_C = 1536

_KERNEL_CACHE = {}


def _build_singlepass(C, NB, nblk):
    """Per-core program, whole capacity resident: phase 1 (stream W1 once)
    then phase 2 (stream W2 once, prefetched on the SWDGE queue)."""
    assert nblk * NB == C

    nc = bacc.Bacc(None, target_bir_lowering=False)
    # Block 0 is split into k-pairs (1456 B per partition per DMA — small
    # runs below ~1 KiB/partition collapse to ~20 GB/s); later blocks are
    # partition-major for single whole-block DMAs.
    xT0 = nc.dram_tensor("xT0", [KD // 2, 128, 2, NB], BF16,
                         kind="ExternalInput")
    xTb = nc.dram_tensor("xTb", [nblk - 1, 128, KD, NB], BF16,
                         kind="ExternalInput")
    w1 = nc.dram_tensor("w1", [MF, 128, KD, 128], BF16, kind="ExternalInput")
    b1c = nc.dram_tensor("b1c", [128, MF], F32, kind="ExternalInput")
    w2 = nc.dram_tensor("w2", [MD, 128, KF, 128], BF16, kind="ExternalInput")
    b2c = nc.dram_tensor("b2c", [128, MD], F32, kind="ExternalInput")
    yT = nc.dram_tensor("yT", [D, C], BF16, kind="ExternalOutput")

    with TileContext(nc) as tc:
        with (
            tc.tile_pool(name="acts", bufs=1) as acts,
            tc.tile_pool(name="wpool", bufs=1) as wpool,
            tc.tile_pool(name="cpool", bufs=1) as cpool,
            tc.tile_pool(name="opool", bufs=1) as opool,
            tc.tile_pool(name="psum", bufs=8, space="PSUM") as psum,
        ):
            # biases via gpsimd (SWDGE) — keeps both HWDGE queues free for
            # the latency-critical streams.
            b1t = cpool.tile([128, MF], F32, name="b1t")
            nc.gpsimd.dma_start(out=b1t[:], in_=b1c[:])
            b2t = cpool.tile([128, MD], F32, name="b2t")
            nc.gpsimd.dma_start(out=b2t[:], in_=b2c[:])

            MCH = 4

            # Startup choreography: X block 0 lands as 4 k-pair DMAs
            # interleaved across the scalar and sync HWDGE engines (each
            # dma_start occupies its engine ~650 ns, so one engine alone
            # would gate block 0 on trigger issue). The sync engine leads
            # with the first W1 tiles.
            xp = [None] * (KD // 2)
            w1_pre = []

            def w1_dma(m):
                w1t = wpool.tile([128, KD, 128], BF16, name="w1t",
                                 tag="w1t", bufs=4 * MCH)
                nc.sync.dma_start(out=w1t[:], in_=w1[m])
                return w1t

            def x0_dma(j, eng):
                t = acts.tile([128, 2, NB], BF16, name=f"x0_{j}",
                              tag=f"x0_{j}")
                eng.dma_start(out=t[:], in_=xT0[j])
                xp[j] = t

            w1_pre.append(w1_dma(0))          # sync
            x0_dma(0, nc.scalar)
            x0_dma(1, nc.sync)
            x0_dma(2, nc.scalar)
            x0_dma(3, nc.sync)
            w1_pre.append(w1_dma(1))
            # Blocks 1+ as whole-block DMAs on the scalar queue.
            xbs = []
            for nb in range(1, nblk):
                t = acts.tile([128, KD, NB], BF16, name=f"xb{nb}",
                              tag=f"xb{nb}")
                nc.scalar.dma_start(out=t[:], in_=xTb[nb - 1])
                xbs.append(t)
            w1_pre.append(w1_dma(2))
            w1_pre.append(w1_dma(3))

            def xrhs(nb, k):
                if nb == 0:
                    return xp[k // 2][:, k % 2, :]
                return xbs[nb - 1][:, k, :]

            # W2 destination tiles (SBUF-resident, 8 x 1 MB). Their DMAs
            # are emitted inside the phase-1 chunk loop below, so priority
            # order spreads the W2 stream across phase 1 instead of
            # fighting the startup X burst.
            w2ts = [
                wpool.tile([128, KF, 128], BF16, name=f"w2_{mo}",
                           tag=f"w2_{mo}")
                for mo in range(MD)
            ]

            hts = [
                acts.tile([128, C], BF16, name=f"ht{m}", tag=f"ht{m}")
                for m in range(MF)
            ]

            # phase 1: H^T[m] = relu(sum_k W1[k,m]^T @ X^T[k] + b1[m])
            # W1 tile for m streams as one 256 KB DMA on the sync queue.
            # m advances in chunks with the block loop outside, so early
            # matmuls ride the arriving X stream (block 0 serves a chunk
            # of m-tiles of work before block 1 is needed).
            for mc in range(0, MF, MCH):
                if mc == 0:
                    w1ts = w1_pre
                    mo = None
                else:
                    w1ts = [w1_dma(m) for m in range(mc, mc + MCH)]
                    # One W2 tile streams per chunk (scalar queue), in
                    # quarter pieces paced across the chunk's block loop:
                    # spread evenly over phase 1, clear of the startup
                    # burst and without multi-us full-rate HBM bursts.
                    mo = mc // MCH - 1
                QP = KF // 4
                for nb in range(nblk):
                    if mo is not None:
                        qs = slice(nb * QP, (nb + 1) * QP)
                        nc.scalar.dma_start(
                            out=w2ts[mo][:, qs, :], in_=w2[mo][:, qs, :]
                        )
                    ns = slice(nb * NB, (nb + 1) * NB)
                    for m in range(mc, mc + MCH):
                        ps = psum.tile([128, 512], F32, name="ps",
                                       tag="ps")[:, :NB]
                        for k in range(KD):
                            nc.tensor.matmul(
                                ps, lhsT=w1ts[m - mc][:, k, :],
                                rhs=xrhs(nb, k),
                                start=(k == 0), stop=(k == KD - 1),
                            )
                        nc.scalar.activation(
                            hts[m][:, ns], ps,
                            mybir.ActivationFunctionType.Relu,
                            bias=b1t[:, m:m + 1],
                        )
                if mo is not None:
                    qs = slice(nblk * QP, KF)
                    nc.scalar.dma_start(
                        out=w2ts[mo][:, qs, :], in_=w2[mo][:, qs, :]
                    )

            nc.scalar.dma_start(out=w2ts[MD - 1][:], in_=w2[MD - 1])

            # phase 2: Y^T[mo] = sum_k W2[k,mo]^T @ H^T[k] + b2[mo]
            # One output tile per mo: the three block activations fill it,
            # then a single DMA (2184 B/partition — fat enough for full
            # DMA rate) stores the whole row block.
            for mo in range(MD):
                w2t = w2ts[mo]
                ot = opool.tile([128, C], BF16, name="ot", tag="ot", bufs=2)
                for nb in range(nblk):
                    ns = slice(nb * NB, (nb + 1) * NB)
                    ps = psum.tile([128, 512], F32, name="ps2", tag="ps")[:, :NB]
                    for k in range(KF):
                        nc.tensor.matmul(
                            ps, lhsT=w2t[:, k, :], rhs=hts[k][:, ns],
                            start=(k == 0), stop=(k == KF - 1),
                        )
                    nc.scalar.activation(
                        ot[:, ns], ps,
                        mybir.ActivationFunctionType.Identity,
                        bias=b2t[:, mo:mo + 1],
                    )
                nc.scalar.dma_start(
                    out=yT[mo * 128:(mo + 1) * 128, :], in_=ot[:]
                )
    nc.compile()
    return nc


def _plan(maxc):
    """Pick capacity/tiling. Blocks must be <= 512 (one PSUM bank of f32)."""
    nblk = max(1, -(-maxc // 512))
    NB = max(256, -(-maxc // nblk))
    C = nblk * NB
    return ("single", C, NB, nblk)


def _get_kernel(plan):
    if plan not in _KERNEL_CACHE:
        kind, C, NB, nblk = plan
        _KERNEL_CACHE[plan] = _build_singlepass(C, NB, nblk)
    return _KERNEL_CACHE[plan]


def kernel(x, Wg, bg, W1, b1, W2, b2):
    x = np.asarray(x, dtype=np.float32)
    Wg = np.asarray(Wg, dtype=np.float32)
    bg = np.asarray(bg, dtype=np.float32)
    W1 = np.asarray(W1, dtype=np.float32)
    b1 = np.asarray(b1, dtype=np.float32)
    W2 = np.asarray(W2, dtype=np.float32)
    b2 = np.asarray(b2, dtype=np.float32)

    fsz = x.shape[:-1]
    xf = x.reshape(-1, D)
    n = xf.shape[0]

    # ---- routing (host): gate -> top-2 -> softmax over the top-2 ----
    gate = xf @ Wg + bg                                   # [N, E] f32
    top2 = np.argsort(-gate, axis=-1, kind="stable")[:, :TOPK]   # desc, ties->low idx
    vals = np.take_along_axis(gate, top2, axis=-1)        # [N, 2] sorted desc
    ex = np.exp(vals - vals[:, :1])
    wts = ex / ex.sum(axis=-1, keepdims=True)             # [N, 2] f32

    idx_lists = []
    wt_lists = []
    counts = np.zeros(E, dtype=np.int64)
    for e in range(E):
        tok, slot = np.nonzero(top2 == e)
        idx_lists.append(tok)
        wt_lists.append(wts[tok, slot])
        counts[e] = tok.shape[0]
    maxc = int(counts.max())

    plan = _plan(maxc)
    _, C, NB, nblk = plan
    assert C <= MAX_SINGLEPASS_C, "capacity beyond single-pass SBUF budget"
    nc = _get_kernel(plan)

    # ---- shard: gather tokens + pre-tile weights per expert ----
    in_maps = []
    for e in range(E):
        xe = np.zeros((C, D), dtype=np.float32)
        xe[:counts[e]] = xf[idx_lists[e]]
        xT = xe.T                                           # [D, C]
        xTk = xT.reshape(KD // 2, 2, 128, nblk, NB)
        xT0 = np.ascontiguousarray(
            xTk[:, :, :, 0, :].transpose(0, 2, 1, 3)
        ).astype(BF16NP)                                    # [KD/2,128,2,NB]
        xTb = np.ascontiguousarray(
            xTk[:, :, :, 1:, :].transpose(3, 2, 0, 1, 4)
        ).reshape(nblk - 1, 128, KD, NB).astype(BF16NP)     # [nblk-1,128,KD,NB]
        w1h = np.ascontiguousarray(
            W1[e].reshape(KD, 128, MF, 128).transpose(2, 1, 0, 3)
        ).astype(BF16NP)                                    # [MF,128,KD,128]
        w2h = np.ascontiguousarray(
            W2[e].reshape(KF, 128, MD, 128).transpose(2, 1, 0, 3)
        ).astype(BF16NP)                                    # [MD,128,KF,128]
        b1c = np.ascontiguousarray(b1[e].reshape(MF, 128).T)  # [128, MF]
        b2c = np.ascontiguousarray(b2[e].reshape(MD, 128).T)  # [128, MD]
        in_maps.append(
            {"xT0": xT0, "xTb": xTb, "w1": w1h, "b1c": b1c, "w2": w2h,
             "b2c": b2c}
        )

    res = run_bass_kernel_spmd(nc, in_maps, core_ids=list(range(E)))

    # ---- combine (host): apply top-2 softmax weights, scatter-add ----
    out = np.zeros((n, D), dtype=np.float32)
    for e in range(E):
        ye = res.results[e]["yT"].astype(np.float32).T[:counts[e]]  # [count, D]
        out[idx_lists[e]] += wt_lists[e][:, None] * ye
    return out.reshape(*fsz, D)


# revision 31
# speedup vs baseline: 1.0322x; 1.0017x over previous
"""MoE block (8 experts, top-2) on 8 Trainium2 NeuronCores.

Strategy: expert parallelism. The gate (x @ Wg + bg, 0.01% of total FLOPs)
plus top-2 routing runs on the host as part of the sharding step; each of
the 8 cores then runs one expert's FFN over that expert's tokens:

    yT_e = (relu(X_e @ W1[e] + b1[e]) @ W2[e] + b2[e])^T

Device-side layout keeps activations transposed ([feature, token]) so both
matmuls use natural weight layouts as the stationary operand:

    H^T = W1^T X^T   (contract d=1024,  8 k-tiles)
    Y^T = W2^T H^T   (contract dff=4096, 32 k-tiles)

Operands are bf16 (f32 PSUM accumulation): same 1-col/cycle PE rate as
f32r but LDWEIGHTS gets FWL (hides under the matmul stream) and all HBM
traffic halves. End-to-end rel err ~5e-4, far inside the 2e-2 gate.
X^T and H^T stay resident in SBUF for the whole token capacity while W1
and W2 each stream from HBM exactly once (phase-split). X^T lands as one
DMA per token block so the first matmuls start ~2 us in; W2 streams on
the gpsimd (SWDGE) queue so its prefetch overlaps the tail of the W1
stream. The host applies the top-2 softmax weights and scatters back.
"""

import numpy as np
import ml_dtypes

import concourse.bacc as bacc
import concourse.mybir as mybir
from concourse.tile import TileContext
from concourse.bass_utils import run_bass_kernel_spmd

D = 1024
DFF = 4096
E = 8
TOPK = 2
KD = D // 128      # 8   k-tiles for phase 1
MF = DFF // 128    # 32  dff tiles (phase-1 output / phase-2 contraction)
KF = DFF // 128    # 32
MD = D // 128      # 8   output d tiles

F32 = mybir.dt.float32
BF16 = mybir.dt.bfloat16
BF16NP = ml_dtypes.bfloat16

# Per-partition SBUF: hts 64*C B + xblk 16*C B + resident W2 64 KiB +
# W1 ring/consts ~10 KiB must fit ~208 KiB usable.
MAX_SINGLEPASS_C = 1536

_KERNEL_CACHE = {}


def _build_singlepass(C, NB, nblk):
    """Per-core program, whole capacity resident: phase 1 (stream W1 once)
    then phase 2 (stream W2 once, prefetched on the SWDGE queue)."""
    assert nblk * NB == C

    nc = bacc.Bacc(None, target_bir_lowering=False)
    # Block 0 is split into k-pairs (1456 B per partition per DMA — small
    # runs below ~1 KiB/partition collapse to ~20 GB/s); later blocks are
    # partition-major for single whole-block DMAs.
    xT0 = nc.dram_tensor("xT0", [KD // 2, 128, 2, NB], BF16,
                         kind="ExternalInput")
    xTb = nc.dram_tensor("xTb", [nblk - 1, 128, KD, NB], BF16,
                         kind="ExternalInput")
    w1 = nc.dram_tensor("w1", [MF, 128, KD, 128], BF16, kind="ExternalInput")
    b1c = nc.dram_tensor("b1c", [128, MF], F32, kind="ExternalInput")
    w2 = nc.dram_tensor("w2", [MD, 128, KF, 128], BF16, kind="ExternalInput")
    b2c = nc.dram_tensor("b2c", [128, MD], F32, kind="ExternalInput")
    yT = nc.dram_tensor("yT", [D, C], BF16, kind="ExternalOutput")

    with TileContext(nc) as tc:
        with (
            tc.tile_pool(name="acts", bufs=1) as acts,
            tc.tile_pool(name="wpool", bufs=1) as wpool,
            tc.tile_pool(name="cpool", bufs=1) as cpool,
            tc.tile_pool(name="opool", bufs=1) as opool,
            tc.tile_pool(name="psum", bufs=8, space="PSUM") as psum,
        ):
            # biases via gpsimd (SWDGE) — keeps both HWDGE queues free for
            # the latency-critical streams.
            b1t = cpool.tile([128, MF], F32, name="b1t")
            nc.gpsimd.dma_start(out=b1t[:], in_=b1c[:])
            b2t = cpool.tile([128, MD], F32, name="b2t")
            nc.gpsimd.dma_start(out=b2t[:], in_=b2c[:])

            MCH = 4

            # Startup choreography: X block 0 lands as 4 k-pair DMAs
            # interleaved across the scalar and sync HWDGE engines (each
            # dma_start occupies its engine ~650 ns, so one engine alone
            # would gate block 0 on trigger issue). The sync engine leads
            # with the first W1 tiles.
            xp = [None] * (KD // 2)
            w1_pre = []

            def w1_dma(m):
                w1t = wpool.tile([128, KD, 128], BF16, name="w1t",
                                 tag="w1t", bufs=2 * MCH)
                nc.sync.dma_start(out=w1t[:], in_=w1[m])
                return w1t

            def x0_dma(j, eng):
                t = acts.tile([128, 2, NB], BF16, name=f"x0_{j}",
                              tag=f"x0_{j}")
                eng.dma_start(out=t[:], in_=xT0[j])
                xp[j] = t

            w1_pre.append(w1_dma(0))          # sync
            x0_dma(0, nc.scalar)
            x0_dma(1, nc.sync)
            x0_dma(2, nc.scalar)
            x0_dma(3, nc.sync)
            w1_pre.append(w1_dma(1))
            # Blocks 1+ as whole-block DMAs on the scalar queue.
            xbs = []
            for nb in range(1, nblk):
                t = acts.tile([128, KD, NB], BF16, name=f"xb{nb}",
                              tag=f"xb{nb}")
                nc.scalar.dma_start(out=t[:], in_=xTb[nb - 1])
                xbs.append(t)
            w1_pre.append(w1_dma(2))
            w1_pre.append(w1_dma(3))

            def xrhs(nb, k):
                if nb == 0:
                    return xp[k // 2][:, k % 2, :]
                return xbs[nb - 1][:, k, :]

            # W2 destination tiles (SBUF-resident, 8 x 1 MB). Their DMAs
            # are emitted inside the phase-1 chunk loop below, so priority
            # order spreads the W2 stream across phase 1 instead of
            # fighting the startup X burst.
            w2ts = [
                wpool.tile([128, KF, 128], BF16, name=f"w2_{mo}",
                           tag=f"w2_{mo}")
                for mo in range(MD)
            ]

            hts = [
                acts.tile([128, C], BF16, name=f"ht{m}", tag=f"ht{m}")
                for m in range(MF)
            ]

            # phase 1: H^T[m] = relu(sum_k W1[k,m]^T @ X^T[k] + b1[m])
            # W1 tile for m streams as one 256 KB DMA on the sync queue.
            # m advances in chunks with the block loop outside, so early
            # matmuls ride the arriving X stream (block 0 serves a chunk
            # of m-tiles of work before block 1 is needed).
            for mc in range(0, MF, MCH):
                if mc == 0:
                    w1ts = w1_pre
                    mo = None
                else:
                    w1ts = [w1_dma(m) for m in range(mc, mc + MCH)]
                    # One W2 tile streams per chunk (scalar queue), in
                    # quarter pieces paced across the chunk's block loop:
                    # spread evenly over phase 1, clear of the startup
                    # burst and without multi-us full-rate HBM bursts.
                    mo = mc // MCH - 1
                QP = KF // 4
                for nb in range(nblk):
                    if mo is not None:
                        qs = slice(nb * QP, (nb + 1) * QP)
                        nc.scalar.dma_start(
                            out=w2ts[mo][:, qs, :], in_=w2[mo][:, qs, :]
                        )
                    ns = slice(nb * NB, (nb + 1) * NB)
                    for m in range(mc, mc + MCH):
                        ps = psum.tile([128, 512], F32, name="ps",
                                       tag="ps")[:, :NB]
                        for k in range(KD):
                            nc.tensor.matmul(
                                ps, lhsT=w1ts[m - mc][:, k, :],
                                rhs=xrhs(nb, k),
                                start=(k == 0), stop=(k == KD - 1),
                            )
                        nc.scalar.activation(
                            hts[m][:, ns], ps,
                            mybir.ActivationFunctionType.Relu,
                            bias=b1t[:, m:m + 1],
                        )
                if mo is not None:
                    qs = slice(nblk * QP, KF)
                    nc.scalar.dma_start(
                        out=w2ts[mo][:, qs, :], in_=w2[mo][:, qs, :]
                    )

            nc.scalar.dma_start(out=w2ts[MD - 1][:], in_=w2[MD - 1])

            # phase 2: Y^T[mo] = sum_k W2[k,mo]^T @ H^T[k] + b2[mo]
            # One output tile per mo: the three block activations fill it,
            # then a single DMA (2184 B/partition — fat enough for full
            # DMA rate) stores the whole row block.
            for mo in range(MD):
                w2t = w2ts[mo]
                ot = opool.tile([128, C], BF16, name="ot", tag="ot", bufs=2)
                for nb in range(nblk):
                    ns = slice(nb * NB, (nb + 1) * NB)
                    ps = psum.tile([128, 512], F32, name="ps2", tag="ps")[:, :NB]
                    for k in range(KF):
                        nc.tensor.matmul(
                            ps, lhsT=w2t[:, k, :], rhs=hts[k][:, ns],
                            start=(k == 0), stop=(k == KF - 1),
                        )
                    nc.scalar.activation(
                        ot[:, ns], ps,
                        mybir.ActivationFunctionType.Identity,
                        bias=b2t[:, mo:mo + 1],
                    )
                nc.scalar.dma_start(
                    out=yT[mo * 128:(mo + 1) * 128, :], in_=ot[:]
                )
    nc.compile()
    return nc


def _plan(maxc):
    """Pick capacity/tiling. Blocks must be <= 512 (one PSUM bank of f32)."""
    nblk = max(1, -(-maxc // 512))
    NB = max(256, -(-maxc // nblk))
    C = nblk * NB
    return ("single", C, NB, nblk)


def _get_kernel(plan):
    if plan not in _KERNEL_CACHE:
        kind, C, NB, nblk = plan
        _KERNEL_CACHE[plan] = _build_singlepass(C, NB, nblk)
    return _KERNEL_CACHE[plan]


def kernel(x, Wg, bg, W1, b1, W2, b2):
    x = np.asarray(x, dtype=np.float32)
    Wg = np.asarray(Wg, dtype=np.float32)
    bg = np.asarray(bg, dtype=np.float32)
    W1 = np.asarray(W1, dtype=np.float32)
    b1 = np.asarray(b1, dtype=np.float32)
    W2 = np.asarray(W2, dtype=np.float32)
    b2 = np.asarray(b2, dtype=np.float32)

    fsz = x.shape[:-1]
    xf = x.reshape(-1, D)
    n = xf.shape[0]

    # ---- routing (host): gate -> top-2 -> softmax over the top-2 ----
    gate = xf @ Wg + bg                                   # [N, E] f32
    top2 = np.argsort(-gate, axis=-1, kind="stable")[:, :TOPK]   # desc, ties->low idx
    vals = np.take_along_axis(gate, top2, axis=-1)        # [N, 2] sorted desc
    ex = np.exp(vals - vals[:, :1])
    wts = ex / ex.sum(axis=-1, keepdims=True)             # [N, 2] f32

    idx_lists = []
    wt_lists = []
    counts = np.zeros(E, dtype=np.int64)
    for e in range(E):
        tok, slot = np.nonzero(top2 == e)
        idx_lists.append(tok)
        wt_lists.append(wts[tok, slot])
        counts[e] = tok.shape[0]
    maxc = int(counts.max())

    plan = _plan(maxc)
    _, C, NB, nblk = plan
    assert C <= MAX_SINGLEPASS_C, "capacity beyond single-pass SBUF budget"
    nc = _get_kernel(plan)

    # ---- shard: gather tokens + pre-tile weights per expert ----
    in_maps = []
    for e in range(E):
        xe = np.zeros((C, D), dtype=np.float32)
        xe[:counts[e]] = xf[idx_lists[e]]
        xT = xe.T                                           # [D, C]
        xTk = xT.reshape(KD // 2, 2, 128, nblk, NB)
        xT0 = np.ascontiguousarray(
            xTk[:, :, :, 0, :].transpose(0, 2, 1, 3)
        ).astype(BF16NP)                                    # [KD/2,128,2,NB]
        xTb = np.ascontiguousarray(
            xTk[:, :, :, 1:, :].transpose(3, 2, 0, 1, 4)
        ).reshape(nblk - 1, 128, KD, NB).astype(BF16NP)     # [nblk-1,128,KD,NB]
        w1h = np.ascontiguousarray(
            W1[e].reshape(KD, 128, MF, 128).transpose(2, 1, 0, 3)
        ).astype(BF16NP)                                    # [MF,128,KD,128]
        w2h = np.ascontiguousarray(
            W2[e].reshape(KF, 128, MD, 128).transpose(2, 1, 0, 3)
        ).astype(BF16NP)                                    # [MD,128,KF,128]
        b1c = np.ascontiguousarray(b1[e].reshape(MF, 128).T)  # [128, MF]
        b2c = np.ascontiguousarray(b2[e].reshape(MD, 128).T)  # [128, MD]
        in_maps.append(
            {"xT0": xT0, "xTb": xTb, "w1": w1h, "b1c": b1c, "w2": w2h,
             "b2c": b2c}
        )

    res = run_bass_kernel_spmd(nc, in_maps, core_ids=list(range(E)))

    # ---- combine (host): apply top-2 softmax weights, scatter-add ----
    out = np.zeros((n, D), dtype=np.float32)
    for e in range(E):
        ye = res.results[e]["yT"].astype(np.float32).T[:counts[e]]  # [count, D]
        out[idx_lists[e]] += wt_lists[e][:, None] * ye
    return out.reshape(*fsz, D)
